# revision 1
# baseline (speedup 1.0000x reference)
"""Trainium2 Bass kernel for nn_Block_36575941492917 (ViG / gnn_message_passing).

Data-parallel over batch: 16 images -> 8 cores x 2 images.

Per-image pipeline (all activations kept c-major (C, N) in SBUF):
  1. conv1x1 C->C + folded BN          (PE matmul + ACT bias copyback)
  2. 2x2 avg pool -> Y4 (= 4*Y)        (DVE strided adds)
  3. transpose Y4 tiles, column norms  (PE transpose + ACT square/accum)
     -> yn2 = 2*Y/||Y|| c-major, Yt = Y bf16 gather table in DRAM
  4. per 112-row n-tile: scores s = 2<xn,yn> - 1 - rel_pos
     (PE matmul, ACT scale by 1/||hx_col||, DVE subtract of rel tile)
  5. top-9 via DVE max8 / max_index / match_replace
  6. indirect-DMA gather of neighbor rows from Yt, DVE max over k,
     PE transpose back to c-major, msg = max_k(y_j) - hx
  7. gc conv (2C->2C, interleave folded into host-permuted weights) + BN+GELU
  8. fc2 (2C->C) + BN, residual -> score_map
  9. FFN (C->4C GELU 4C->C, BNs folded) + residual -> out
"""

import numpy as np

import concourse.bass as bass
import concourse.tile as tile
from concourse import bacc, mybir
from concourse.bass import IndirectOffsetOnAxis
from concourse.bass_utils import run_bass_kernel_spmd
from concourse.masks import make_identity

F32 = mybir.dt.float32
BF16 = mybir.dt.bfloat16
U32 = mybir.dt.uint32
AF = mybir.ActivationFunctionType
OP = mybir.AluOpType
AX = mybir.AxisListType

B, C, H, W = 16, 96, 56, 56
N = H * W            # 3136
NR = N // 4          # 784
KNN = 9
NCORES = 8
IPC = B // NCORES    # 2 images per core
NT = 112             # n-tile rows for the knn/topk phase
NTILES = N // NT     # 28
CHK = 448            # n-chunk for conv phases
NCHK = N // CHK      # 7
C2 = 2 * C           # 192
C4 = 4 * C           # 384
EPS = 1e-5
NEG = -1.0e30


def _build_nc(reps: int = 1):
    nc = bacc.Bacc("TRN2", target_bir_lowering=False, debug=False,
                   num_devices=NCORES)

    # ---- DRAM I/O ----
    xs = nc.dram_tensor("xs", [IPC, C, N], F32, kind="ExternalInput")
    rel = nc.dram_tensor("rel", [N, NR], F32, kind="ExternalInput")
    w1t = nc.dram_tensor("w1t", [C, C], F32, kind="ExternalInput")
    b1 = nc.dram_tensor("b1", [C, 1], F32, kind="ExternalInput")
    w2ta = nc.dram_tensor("w2ta", [C, C2], F32, kind="ExternalInput")
    w2tb = nc.dram_tensor("w2tb", [C, C2], F32, kind="ExternalInput")
    b2 = nc.dram_tensor("b2", [C, 2], F32, kind="ExternalInput")
    w3t = nc.dram_tensor("w3t", [C, 2 * C], F32, kind="ExternalInput")
    b3 = nc.dram_tensor("b3", [C, 1], F32, kind="ExternalInput")
    w4t = nc.dram_tensor("w4t", [C, C4], F32, kind="ExternalInput")
    b4 = nc.dram_tensor("b4", [128, 3], F32, kind="ExternalInput")
    w5t = nc.dram_tensor("w5t", [128, 3 * C], F32, kind="ExternalInput")
    b5 = nc.dram_tensor("b5", [C, 1], F32, kind="ExternalInput")
    out_d = nc.dram_tensor("out", [IPC, C, N], F32, kind="ExternalOutput")
    # internal gather tables (bf16 rows = reduced-graph feature vectors)
    yts = [nc.dram_tensor(f"yt{i}", [NR, C], BF16) for i in range(IPC)]

    with tile.TileContext(nc) as tc:
        _emit(nc, tc, reps, xs, rel, w1t, b1, w2ta, w2tb, b2, w3t, b3,
              w4t, b4, w5t, b5, out_d, yts)
    nc.compile()
    return nc


def _emit(nc, tc, reps, xs, rel, w1t, b1, w2ta, w2tb, b2, w3t, b3,
          w4t, b4, w5t, b5, out_d, yts):
    from contextlib import ExitStack
    ctx = ExitStack()
    with ctx:
        singles = ctx.enter_context(tc.tile_pool(name="singles", bufs=1))
        resid = ctx.enter_context(tc.tile_pool(name="resid", bufs=1))

        # identities for PE transposes
        id_f32 = singles.tile([128, 128], F32, tag="id_f32")
        make_identity(nc, id_f32)
        id_bf16 = singles.tile([128, 128], BF16, tag="id_bf16")
        make_identity(nc, id_bf16)
        neg1 = singles.tile([128, 1], F32, tag="neg1")
        nc.vector.memset(neg1[:], -1.0)

        # weights -> SBUF
        def load(name, dram, shape, dt=F32):
            t = singles.tile(shape, dt, tag=name)
            nc.sync.dma_start(out=t[:], in_=dram[:])
            return t

        w1t_s = load("w1t", w1t, [C, C])
        b1_s = load("b1", b1, [C, 1])
        w2ta_s = load("w2ta", w2ta, [C, C2])
        w2tb_s = load("w2tb", w2tb, [C, C2])
        b2_s = load("b2", b2, [C, 2])
        w3t_s = load("w3t", w3t, [C, 2 * C])
        b3_s = load("b3", b3, [C, 1])
        w4t_s = load("w4t", w4t, [C, C4])
        b4_s = load("b4", b4, [128, 3])
        w5t_s = load("w5t", w5t, [128, 3 * C])
        b5_s = load("b5", b5, [C, 1])

        # persistent per-image activations (c-major)
        X = [resid.tile([C, N], F32, tag=f"X{i}", name=f"X{i}")
             for i in range(IPC)]
        Hx = [resid.tile([C, N], F32, tag=f"Hx{i}", name=f"Hx{i}")
              for i in range(IPC)]
        Msg = [resid.tile([C, N], F32, tag=f"Msg{i}", name=f"Msg{i}")
               for i in range(IPC)]
        Smap = [resid.tile([C, N], F32, tag=f"S{i}", name=f"S{i}")
                for i in range(IPC)]
        Yn2 = [resid.tile([C, NR], F32, tag=f"Yn2{i}", name=f"Yn2{i}")
               for i in range(IPC)]

        def body(_iv=None):
            # ---------------- phase A/B/C/D: conv1, pool, normalize -------
            with (
                tc.tile_pool(name="ptmp", bufs=2) as ptmp,
                tc.tile_pool(name="psA", bufs=2, space="PSUM") as psA,
                tc.tile_pool(name="psB", bufs=2, space="PSUM") as psB,
            ):
                for i in range(IPC):
                    nc.sync.dma_start(out=X[i][:], in_=xs[i, :, :])
                    # conv1 + BN fold
                    for ch in range(NCHK):
                        sl = bass.ts(ch, CHK)
                        ps = psA.tile([C, CHK], F32, tag="conv1")
                        nc.tensor.matmul(ps[:], lhsT=w1t_s[:], rhs=X[i][:, sl],
                                         start=True, stop=True)
                        nc.scalar.activation(Hx[i][:, sl], ps[:], AF.Identity,
                                             bias=b1_s[:, 0:1], scale=1.0)
                    # 2x2 avg pool (x4)
                    t1 = ptmp.tile([C, N // 2], F32, tag="t1")
                    hv = Hx[i].rearrange("p (x two) -> p x two", two=2)
                    nc.vector.tensor_tensor(t1[:], hv[:, :, 0], hv[:, :, 1],
                                            op=OP.add)
                    y4 = ptmp.tile([C, NR], F32, tag="y4")
                    tv = t1.rearrange("p (h two w) -> p h two w", two=2, w=28)
                    nc.vector.tensor_tensor(y4[:], tv[:, :, 0, :], tv[:, :, 1, :],
                                            op=OP.add)
                    # per-m-column norms + write gather table + yn2
                    for mt in range(NR // NT):  # 7 tiles of 112
                        msl = bass.ts(mt, NT)
                        pt = psA.tile([NT, C], F32, tag="ytr")
                        nc.tensor.transpose(pt[:], y4[:, msl], id_f32[:C, :C])
                        ytb = ptmp.tile([NT, C], BF16, tag="ytb")
                        nc.scalar.activation(ytb[:], pt[:], AF.Copy, bias=0.0,
                                             scale=0.25)
                        nc.sync.dma_start(out=yts[i][msl, :], in_=ytb[:])
                        sq = ptmp.tile([NT, C], F32, tag="sq")
                        ssq = ptmp.tile([NT, 1], F32, tag="ssq")
                        nc.scalar.activation(sq[:], pt[:], AF.Square,
                                             accum_out=ssq[:])
                        rt = ptmp.tile([NT, 1], F32, tag="rt")
                        nc.scalar.activation(rt[:], ssq[:], AF.Sqrt, scale=0.25)
                        rec = ptmp.tile([NT, 1], F32, tag="rec")
                        nc.vector.reciprocal(rec[:], rt[:])
                        ynt = ptmp.tile([NT, C], F32, tag="ynt")
                        nc.scalar.activation(ynt[:], pt[:], AF.Copy, bias=0.0,
                                             scale=rec[:])
                        pb = psB.tile([C, NT], F32, tag="ynb")
                        nc.tensor.transpose(pb[:], ynt[:], id_f32[:NT, :NT])
                        nc.scalar.activation(Yn2[i][:, msl], pb[:], AF.Copy,
                                             bias=0.0, scale=1.0)

            # ---------------- phase E: scores + top-9 + gather + msg ------
            with (
                tc.tile_pool(name="relp", bufs=2) as relp,
                tc.tile_pool(name="sp", bufs=2) as sp,
                tc.tile_pool(name="ip", bufs=3) as ip,
                tc.tile_pool(name="gp", bufs=2) as gp,
                tc.tile_pool(name="psS", bufs=2, space="PSUM") as psS,
                tc.tile_pool(name="psT", bufs=2, space="PSUM") as psT,
                tc.tile_pool(name="psM", bufs=2, space="PSUM") as psM,
            ):
                for nt in range(NTILES):
                    nsl = bass.ts(nt, NT)
                    rel_t = relp.tile([NT, NR], F32, tag="rel")
                    nc.sync.dma_start(out=rel_t[:], in_=rel[nsl, :])
                    for i in range(IPC):
                        # 1/||hx col||
                        pht = psT.tile([NT, C], F32, tag="ht")
                        nc.tensor.transpose(pht[:], Hx[i][:, nsl],
                                            id_f32[:C, :C])
                        hsq = ip.tile([NT, C], F32, tag="hsq")
                        hssq = ip.tile([NT, 1], F32, tag="hssq")
                        nc.scalar.activation(hsq[:], pht[:], AF.Square,
                                             accum_out=hssq[:])
                        hrt = ip.tile([NT, 1], F32, tag="hrt")
                        nc.scalar.activation(hrt[:], hssq[:], AF.Sqrt)
                        invr = ip.tile([NT, 1], F32, tag="invr")
                        nc.vector.reciprocal(invr[:], hrt[:])
                        # scores
                        ps = psS.tile([NT, NR], F32, tag="s")
                        nc.tensor.matmul(ps[:, 0:512], lhsT=Hx[i][:, nsl],
                                         rhs=Yn2[i][:, 0:512],
                                         start=True, stop=True)
                        nc.tensor.matmul(ps[:, 512:NR], lhsT=Hx[i][:, nsl],
                                         rhs=Yn2[i][:, 512:NR],
                                         start=True, stop=True)
                        s = sp.tile([NT, NR], F32, tag="s")
                        nc.scalar.activation(s[:], ps[:], AF.Identity,
                                             bias=neg1[:NT, 0:1], scale=invr[:])
                        nc.vector.tensor_tensor(s[:], s[:], rel_t[:],
                                                op=OP.subtract)
                        # top-9
                        m8 = ip.tile([NT, 8], F32, tag="m8")
                        nc.vector.max(m8[:], s[:])
                        i8 = ip.tile([NT, 8], U32, tag="i8")
                        nc.vector.max_index(i8[:], m8[:], s[:])
                        srep = sp.tile([NT, NR], F32, tag="srep")
                        nc.vector.match_replace(srep[:], in_to_replace=m8[:],
                                                in_values=s[:], imm_value=NEG)
                        m8b = ip.tile([NT, 8], F32, tag="m8b")
                        nc.vector.max(m8b[:], srep[:])
                        v9 = ip.tile([NT, 8], F32, tag="v9")
                        nc.vector.tensor_copy(v9[:],
                                              m8b[:, 0:1].to_broadcast([NT, 8]))
                        i9 = ip.tile([NT, 8], U32, tag="i9")
                        nc.vector.max_index(i9[:], v9[:], s[:])
                        # gather 9 neighbor rows (bf16) from DRAM table
                        g = gp.tile([NT, KNN, C], BF16, tag="g")
                        for k in range(8):
                            nc.gpsimd.indirect_dma_start(
                                out=g[:, k, :], out_offset=None,
                                in_=yts[i][:],
                                in_offset=IndirectOffsetOnAxis(
                                    ap=i8[:, k:k + 1], axis=0))
                        nc.gpsimd.indirect_dma_start(
                            out=g[:, 8, :], out_offset=None,
                            in_=yts[i][:],
                            in_offset=IndirectOffsetOnAxis(
                                ap=i9[:, 0:1], axis=0))
                        # max over k, transpose to c-major, msg = max - hx
                        mx = gp.tile([NT, C], BF16, tag="mx")
                        gv = g.rearrange("p k c -> p c k")
                        nc.vector.tensor_reduce(mx[:], gv[:, :, :], axis=AX.X,
                                                op=OP.max)
                        pmt = psM.tile([C, NT], BF16, tag="mt")
                        nc.tensor.transpose(pmt[:], mx[:], id_bf16[:NT, :NT])
                        nc.vector.tensor_tensor(Msg[i][:, nsl], pmt[:],
                                                Hx[i][:, nsl], op=OP.subtract)

            # ---------------- phase F/G: gc conv, fc2, FFN ----------------
            with (
                tc.tile_pool(name="ctmp", bufs=2) as ctmp,
                tc.tile_pool(name="psG", bufs=2, space="PSUM") as psG,
                tc.tile_pool(name="psF", bufs=2, space="PSUM") as psF,
                tc.tile_pool(name="psU", bufs=2, space="PSUM") as psU,
                tc.tile_pool(name="psV", bufs=2, space="PSUM") as psV,
            ):
                for i in range(IPC):
                    for ch in range(NCHK):
                        sl = bass.ts(ch, CHK)
                        # gc conv: out 192 ch in two groups of 96
                        g1 = ctmp.tile([C, 2, CHK], F32, tag="g1")
                        for gi in range(2):
                            gsl = bass.ts(gi, C)
                            pg = psG.tile([C, CHK], F32, tag="pg")
                            nc.tensor.matmul(pg[:], lhsT=w2ta_s[:, gsl],
                                             rhs=Hx[i][:, sl],
                                             start=True, stop=False)
                            nc.tensor.matmul(pg[:], lhsT=w2tb_s[:, gsl],
                                             rhs=Msg[i][:, sl],
                                             start=False, stop=True)
                            nc.scalar.activation(g1[:, gi, :], pg[:], AF.Gelu,
                                                 bias=b2_s[:, gi:gi + 1])
                        # fc2 + residual -> score map
                        pf = psF.tile([C, CHK], F32, tag="pf")
                        nc.tensor.matmul(pf[:], lhsT=w3t_s[:, 0:C],
                                         rhs=g1[:, 0, :], start=True, stop=False)
                        nc.tensor.matmul(pf[:], lhsT=w3t_s[:, C:2 * C],
                                         rhs=g1[:, 1, :], start=False, stop=True)
                        t3 = ctmp.tile([C, CHK], F32, tag="t3")
                        nc.scalar.activation(t3[:], pf[:], AF.Identity,
                                             bias=b3_s[:, 0:1])
                        nc.vector.tensor_tensor(Smap[i][:, sl], t3[:],
                                                X[i][:, sl], op=OP.add)
                        # FFN
                        u = ctmp.tile([128, 3, CHK], F32, tag="u")
                        for gi in range(3):
                            pu = psU.tile([128, CHK], F32, tag="pu")
                            nc.tensor.matmul(pu[:], lhsT=w4t_s[:, bass.ts(gi, 128)],
                                             rhs=Smap[i][:, sl],
                                             start=True, stop=True)
                            nc.scalar.activation(u[:, gi, :], pu[:], AF.Gelu,
                                                 bias=b4_s[:, gi:gi + 1])
                        pv = psV.tile([C, CHK], F32, tag="pv")
                        for gi in range(3):
                            nc.tensor.matmul(pv[:], lhsT=w5t_s[:, bass.ts(gi, C)],
                                             rhs=u[:, gi, :],
                                             start=(gi == 0), stop=(gi == 2))
                        t5 = ctmp.tile([C, CHK], F32, tag="t5")
                        nc.scalar.activation(t5[:], pv[:], AF.Identity,
                                             bias=b5_s[:, 0:1])
                        ot = ctmp.tile([C, CHK], F32, tag="ot")
                        nc.vector.tensor_tensor(ot[:], t5[:], Smap[i][:, sl],
                                                op=OP.add)
                        nc.sync.dma_start(out=out_d[i, :, sl], in_=ot[:])

        if reps == 1:
            body()
        else:
            with tc.For_i(0, reps, 1) as iv:
                body(iv)


# ------------------------- host side ---------------------------------------

def _fold_bn(g, b, m, v):
    inv = g / np.sqrt(v + EPS)
    return inv, b - m * inv


def _prep_weights(inp):
    f32 = np.float32
    o = {}
    inv1, sh1 = _fold_bn(inp["g_bn1_g"], inp["g_bn1_b"], inp["g_bn1_m"],
                         inp["g_bn1_v"])
    w1 = inp["g_fc1_w"] * inv1[:, None]
    b1 = inp["g_fc1_b"] * inv1 + sh1
    o["w1t"] = np.ascontiguousarray(w1.T, f32)
    o["b1"] = np.ascontiguousarray(b1[:, None], f32)

    inv2, sh2 = _fold_bn(inp["gc_bn_g"], inp["gc_bn_b"], inp["gc_bn_m"],
                         inp["gc_bn_v"])
    w2 = inp["gc_w"] * inv2[:, None]
    b2v = inp["gc_b"] * inv2 + sh2
    perm = np.concatenate([np.arange(0, C2, 2), np.arange(1, C2, 2)])
    w2p = w2[:, perm]          # stacked [hx; msg] input order
    w2T = w2p.T                # (192 in, 192 out)
    o["w2ta"] = np.ascontiguousarray(w2T[:C, :], f32)
    o["w2tb"] = np.ascontiguousarray(w2T[C:, :], f32)
    o["b2"] = np.ascontiguousarray(
        np.stack([b2v[:C], b2v[C:]], axis=1), f32)

    inv3, sh3 = _fold_bn(inp["g_bn2_g"], inp["g_bn2_b"], inp["g_bn2_m"],
                         inp["g_bn2_v"])
    w3 = inp["g_fc2_w"] * inv3[:, None]    # (96, 192)
    b3v = inp["g_fc2_b"] * inv3 + sh3
    w3T = w3.T                              # (192, 96)
    o["w3t"] = np.ascontiguousarray(
        np.concatenate([w3T[:C, :], w3T[C:, :]], axis=1), f32)  # (96, 192)
    o["b3"] = np.ascontiguousarray(b3v[:, None], f32)

    inv4, sh4 = _fold_bn(inp["f_bn1_g"], inp["f_bn1_b"], inp["f_bn1_m"],
                         inp["f_bn1_v"])
    w4 = inp["f_fc1_w"] * inv4[:, None]    # (384, 96)
    b4v = inp["f_fc1_b"] * inv4 + sh4
    o["w4t"] = np.ascontiguousarray(w4.T, f32)   # (96, 384)
    o["b4"] = np.ascontiguousarray(b4v.reshape(3, 128).T, f32)  # (128, 3)

    inv5, sh5 = _fold_bn(inp["f_bn2_g"], inp["f_bn2_b"], inp["f_bn2_m"],
                         inp["f_bn2_v"])
    w5 = inp["f_fc2_w"] * inv5[:, None]    # (96, 384)
    b5v = inp["f_fc2_b"] * inv5 + sh5
    w5T = w5.T                              # (384, 96)
    o["w5t"] = np.ascontiguousarray(
        np.concatenate([w5T[gi * 128:(gi + 1) * 128, :] for gi in range(3)],
                       axis=1), f32)        # (128, 288)
    o["b5"] = np.ascontiguousarray(b5v[:, None], f32)
    return o


_NC_CACHE = {}


def get_nc(reps: int = 1):
    if reps not in _NC_CACHE:
        _NC_CACHE[reps] = _build_nc(reps)
    return _NC_CACHE[reps]


def run(inputs, reps: int = 1):
    nc = get_nc(reps)
    wts = _prep_weights({k: np.asarray(v) for k, v in inputs.items()})
    x = np.asarray(inputs["x"], np.float32).reshape(B, C, N)
    relf = np.ascontiguousarray(
        np.asarray(inputs["rel_pos"], np.float32).reshape(N, NR))
    in_maps = []
    for c in range(NCORES):
        m = {"xs": np.ascontiguousarray(x[c * IPC:(c + 1) * IPC]),
             "rel": relf}
        m.update(wts)
        in_maps.append(m)
    res = run_bass_kernel_spmd(nc, in_maps, list(range(NCORES)))
    out = np.concatenate([res.results[c]["out"] for c in range(NCORES)],
                         axis=0)
    return out.reshape(B, C, H, W)


def kernel(**inputs) -> np.ndarray:
    return run(inputs, reps=1)

